# Initial kernel scaffold
#
"""Trainium2 Bass kernel for masked sparse attention (nn_Attention_86131274154152).

Strategy:
  - 8 heads -> 1 head per NeuronCore (tensor parallel over heads).
  - Host: sort rows by label. The post-softmax label-equality mask then only
    passes keys inside a narrow sorted band around each query tile's label
    range, so the masked attn@V contraction touches <=3 key tiles per query
    tile. The softmax denominator still needs unmasked row sums over all 4096
    keys; those come for free from the ScalarE activation accum_out while
    computing exp.
  - Device per core (head h):
      qkv:   Q.T/K.T/V.T [96,4096] = W.T @ x.T  (fp32r matmuls, W stationary)
      V:     PE-transpose V.T -> V [4096,96] (bf16)
      per query tile t (128 rows):
        S[i,j] tile = Q.T_t.T @ K.T        (bf16, PSUM f32, j in 3 chunks)
        attn = exp(S) (ScalarE, accum_out -> row sums), bf16
        band: attnm = (lab_j == lab_i) * attn  (one DVE scalar_tensor_tensor)
        PE-transpose band blocks, O.T = V_jt.T-style matmuls (bf16)
        final = O.T.T @ W_out_h (fp32r), scaled by 1/rowsum, DMA out
  - Host: sum the 8 per-head partial outputs, undo the sort permutation.
"""

import sys
from contextlib import ExitStack

import numpy as np

for _p in ("/opt/trn_rl_repo", "/root/.axon_site/_ro/trn_rl_repo"):
    if _p not in sys.path:
        sys.path.append(_p)

import concourse.bacc as bacc
import concourse.mybir as mybir
import concourse.tile as tile
from concourse.bass_utils import run_bass_kernel_spmd
from concourse.masks import make_identity

B = 4096
DIM = 768
HEADS = 8
DH = 96
P = 128
KT = 6            # k-tiles over DIM for the qkv projections
NT = B // P       # 32 query tiles
CHUNKS = ((0, 1536), (1536, 1536), (3072, 1024))  # key-dim chunks for dots/exp

F32 = mybir.dt.float32
F32R = mybir.dt.float32r
BF16 = mybir.dt.bfloat16


def _r(ap):
    return ap.bitcast(F32R)


def build_program(bands, max_njt):
    nc = bacc.Bacc("TRN2", target_bir_lowering=False, debug=False)

    xT_d = nc.dram_tensor("xT", [DIM, B], F32, kind="ExternalInput").ap()
    wq_d = nc.dram_tensor("wq", [DIM, DH], F32, kind="ExternalInput").ap()
    wk_d = nc.dram_tensor("wk", [DIM, DH], F32, kind="ExternalInput").ap()
    wv_d = nc.dram_tensor("wv", [DIM, DH], F32, kind="ExternalInput").ap()
    wo_d = nc.dram_tensor("wo", [DH, DIM], F32, kind="ExternalInput").ap()
    lab_row_d = nc.dram_tensor("lab_row", [1, B], F32, kind="ExternalInput").ap()
    lab_col_d = nc.dram_tensor("lab_col", [P, NT], F32, kind="ExternalInput").ap()
    out_d = nc.dram_tensor("out", [B, DIM], F32, kind="ExternalOutput").ap()

    bandw = max_njt * P

    with tile.TileContext(nc) as tc, ExitStack() as top:
        persist = top.enter_context(tc.tile_pool(name="persist", bufs=1))

        wq_sb = persist.tile([P, KT, DH], F32)
        wk_sb = persist.tile([P, KT, DH], F32)
        wv_sb = persist.tile([P, KT, DH], F32)
        for w_sb, w_d in ((wq_sb, wq_d), (wk_sb, wk_d), (wv_sb, wv_d)):
            nc.sync.dma_start(w_sb[:], w_d.rearrange("(kt p) d -> p kt d", p=P))
        wo_sb = persist.tile([DH, DIM], F32)
        nc.sync.dma_start(wo_sb[:], wo_d)
        lab_row_sb = persist.tile([1, B], F32)
        nc.sync.dma_start(lab_row_sb[:], lab_row_d)
        lab_col_sb = persist.tile([P, NT], F32)
        nc.sync.dma_start(lab_col_sb[:], lab_col_d)

        ident_bf = persist.tile([P, P], BF16)
        make_identity(nc, ident_bf)
        ones_sb = persist.tile([1, P], F32)
        nc.vector.memset(ones_sb[:], 1.0)

        qT_sb = persist.tile([DH, B], BF16)
        kT_sb = persist.tile([DH, B], BF16)
        lab_rep = persist.tile([P, B], F32)
        vn_sb = persist.tile([P, NT, DH], BF16)

        # ---- phase A: qkv projections + label broadcast ----
        with (
            tc.tile_pool(name="xa", bufs=KT) as xpool,
            tc.tile_pool(name="vt", bufs=1) as vtpool,
            tc.tile_pool(name="pa", bufs=2, space="PSUM") as apsum,
        ):
            vT_sb = vtpool.tile([DH, B], BF16)
            xts = []
            for kt in range(KT):
                xt = xpool.tile([P, B], F32, tag="xt", name=f"xt{kt}")
                nc.sync.dma_start(xt[:], xT_d[kt * P : (kt + 1) * P, :])
                xts.append(xt)

            # broadcast sorted labels across partitions via ones-matmul
            for c in range(B // 512):
                bc = apsum.tile([P, 512], F32, tag="bc", name=f"bc{c}")
                nc.tensor.matmul(
                    bc[:], _r(ones_sb[:]), _r(lab_row_sb[:, c * 512 : (c + 1) * 512])
                )
                nc.vector.tensor_copy(lab_rep[:, c * 512 : (c + 1) * 512], bc[:])

            for w_sb, dst in ((wq_sb, qT_sb), (wk_sb, kT_sb), (wv_sb, vT_sb)):
                for c in range(B // 512):
                    ps = apsum.tile([DH, 512], F32, tag="qkv", name="qkvps")
                    for kt in range(KT):
                        nc.tensor.matmul(
                            ps[:],
                            _r(w_sb[:, kt, :]),
                            _r(xts[kt][:, c * 512 : (c + 1) * 512]),
                            start=(kt == 0),
                            stop=(kt == KT - 1),
                        )
                    nc.vector.tensor_copy(dst[:, c * 512 : (c + 1) * 512], ps[:])

            # ---- phase A2: V.T -> V natural layout (bf16) ----
            with tc.tile_pool(name="pa2", bufs=2, space="PSUM") as a2psum:
                for jt in range(NT):
                    tpv = a2psum.tile([P, DH], BF16, tag="vt", name=f"vt{jt}")
                    nc.tensor.transpose(
                        tpv[:], vT_sb[:, jt * P : (jt + 1) * P], ident_bf[:DH, :DH]
                    )
                    nc.vector.tensor_copy(vn_sb[:, jt, :], tpv[:])

        # ---- phase B: attention per query tile ----
        with (
            tc.tile_pool(name="attn", bufs=2) as attnpool,
            tc.tile_pool(name="small", bufs=2) as smallpool,
            tc.tile_pool(name="osb", bufs=2) as opool,
            tc.tile_pool(name="ps_s", bufs=2, space="PSUM") as spool,
            tc.tile_pool(name="ps_tp", bufs=1, space="PSUM") as tppool,
            tc.tile_pool(name="ps_m", bufs=1, space="PSUM") as mpool,
        ):
            for t in range(NT):
                jt0, njt = bands[t]
                bw = njt * P
                attn = attnpool.tile([P, B], BF16, tag="attn", name=f"attn{t}")
                acc = smallpool.tile([P, 4], F32, tag="acc", name=f"acc{t}")
                for ci, (c0, cw) in enumerate(CHUNKS):
                    s = spool.tile([P, 1536], F32, tag="s", name=f"s{t}_{ci}")
                    for mi in range(cw // 512):
                        nc.tensor.matmul(
                            s[:, mi * 512 : (mi + 1) * 512],
                            qT_sb[:, t * P : (t + 1) * P],
                            kT_sb[:, c0 + mi * 512 : c0 + (mi + 1) * 512],
                        )
                    nc.scalar.activation(
                        out=attn[:, c0 : c0 + cw],
                        in_=s[:, :cw],
                        func=mybir.ActivationFunctionType.Exp,
                        accum_out=acc[:, ci : ci + 1],
                    )

                sums = smallpool.tile([P, 1], F32, tag="sums", name=f"sums{t}")
                nc.vector.tensor_reduce(
                    out=sums[:],
                    in_=acc[:, : len(CHUNKS)],
                    axis=mybir.AxisListType.X,
                    op=mybir.AluOpType.add,
                )
                recip = smallpool.tile([P, 1], F32, tag="recip", name=f"recip{t}")
                nc.vector.reciprocal(recip[:], sums[:])

                # mask the band: attnm = (lab_j == lab_i) * attn
                attnm = smallpool.tile([P, bandw], BF16, tag="attnm", name=f"am{t}")
                nc.vector.scalar_tensor_tensor(
                    out=attnm[:, :bw],
                    in0=lab_rep[:, jt0 * P : jt0 * P + bw],
                    scalar=lab_col_sb[:, t : t + 1],
                    in1=attn[:, jt0 * P : jt0 * P + bw],
                    op0=mybir.AluOpType.is_equal,
                    op1=mybir.AluOpType.mult,
                )

                tp = tppool.tile([P, bandw], BF16, tag="tp", name=f"tp{t}")
                for b in range(njt):
                    nc.tensor.transpose(
                        tp[:, b * P : (b + 1) * P],
                        attnm[:, b * P : (b + 1) * P],
                        ident_bf[:],
                    )
                attnT = smallpool.tile([P, bandw], BF16, tag="attnT", name=f"aT{t}")
                nc.vector.tensor_copy(attnT[:, :bw], tp[:, :bw])

                ot = mpool.tile([DH, P], F32, tag="m", name=f"ot{t}")
                for b in range(njt):
                    nc.tensor.matmul(
                        ot[:],
                        vn_sb[:, jt0 + b, :],
                        attnT[:, b * P : (b + 1) * P],
                        start=(b == 0),
                        stop=(b == njt - 1),
                    )
                ot_sb = smallpool.tile([DH, P], F32, tag="ot_sb", name=f"os{t}")
                nc.vector.tensor_copy(ot_sb[:], ot[:])

                osb = opool.tile([P, DIM], F32, tag="osb", name=f"o{t}")
                for h in range(2):
                    fin = mpool.tile([P, 384], F32, tag="m", name=f"fin{t}_{h}")
                    nc.tensor.matmul(
                        fin[:],
                        _r(ot_sb[:]),
                        _r(wo_sb[:, h * 384 : (h + 1) * 384]),
                    )
                    nc.vector.tensor_scalar(
                        out=osb[:, h * 384 : (h + 1) * 384],
                        in0=fin[:],
                        scalar1=recip[:],
                        scalar2=None,
                        op0=mybir.AluOpType.mult,
                    )
                nc.sync.dma_start(out_d[t * P : (t + 1) * P, :], osb[:])

    nc.compile()
    return nc


def _host_prep(x, labels, W_qkv, W_out):
    x = np.asarray(x, dtype=np.float32)
    labels = np.asarray(labels)
    W_qkv = np.asarray(W_qkv, dtype=np.float32)
    W_out = np.asarray(W_out, dtype=np.float32)

    perm = np.argsort(labels, kind="stable")
    ls = labels[perm]
    xT = np.ascontiguousarray(x[perm].T)

    lsf = ls.astype(np.float32)
    lab_row = np.ascontiguousarray(lsf.reshape(1, B))
    lab_col = np.ascontiguousarray(lsf.reshape(NT, P).T)

    bands = []
    for t in range(NT):
        jlo = int(np.searchsorted(ls, ls[t * P], side="left"))
        jhi = int(np.searchsorted(ls, ls[t * P + P - 1], side="right"))
        jt0 = jlo // P
        njt = (jhi - 1) // P - jt0 + 1
        bands.append((jt0, njt))
    max_njt = max(n for _, n in bands)

    scale = DH ** -0.5
    in_maps = []
    for h in range(HEADS):
        sl = slice(h * DH, (h + 1) * DH)
        in_maps.append(
            {
                "xT": xT,
                "wq": np.ascontiguousarray(W_qkv[:, sl]) * scale,
                "wk": np.ascontiguousarray(W_qkv[:, 768 + h * DH : 768 + (h + 1) * DH]),
                "wv": np.ascontiguousarray(
                    W_qkv[:, 1536 + h * DH : 1536 + (h + 1) * DH]
                ),
                "wo": np.ascontiguousarray(W_out[sl, :]),
                "lab_row": lab_row,
                "lab_col": lab_col,
            }
        )
    return in_maps, bands, max_njt, perm


_CACHE = {}


def kernel(x, labels, W_qkv, W_out, _trace=False):
    in_maps, bands, max_njt, perm = _host_prep(x, labels, W_qkv, W_out)

    key = (tuple(bands), max_njt)
    if key not in _CACHE:
        _CACHE[key] = build_program(bands, max_njt)
    nc = _CACHE[key]

    res = run_bass_kernel_spmd(
        nc, in_maps, core_ids=list(range(HEADS)), trace=_trace
    )
    acc = np.zeros((B, DIM), dtype=np.float64)
    for c in range(HEADS):
        acc += res.results[c]["out"].astype(np.float64)

    out = np.empty((B, DIM), dtype=np.float32)
    out[perm] = acc.astype(np.float32)
    if _trace:
        return out, res
    return out


# revision 15
# speedup vs baseline: 1.8891x; 1.8891x over previous
"""Trainium2 Bass kernel for masked sparse attention (nn_Attention_86131274154152).

Strategy:
  - 8 heads -> 1 head per NeuronCore (tensor parallel over heads).
  - Host: sort rows by label. The post-softmax label-equality mask then only
    passes keys inside a narrow sorted band around each query tile's label
    range, so the masked attn@V contraction touches <=3 key tiles per query
    tile. The softmax denominator still needs unmasked row sums over all 4096
    keys; those come for free from the ScalarE activation accum_out while
    computing exp.
  - Device per core (head h):
      qkv:   K.T/Q.T/V.T [96,4096] = W.T @ x.T  (fp32r matmuls, W stationary),
             produced column-chunk-major so attention starts early
      V:     PE-transpose V.T -> V [4096,96] (bf16)
      per query tile t (128 rows):
        S[i,j] tile = Q.T_t.T @ K.T        (bf16, PSUM f32, j in 3 chunks)
        attn = exp(S) (ScalarE, accum_out -> row sums), bf16
        band: attnm = (lab_j == lab_i) * attn  (one DVE scalar_tensor_tensor)
        PE-transpose band blocks, O.T = V_jt-stationary matmuls (bf16)
        final = O.T.T @ W_out_h (fp32r), scaled by 1/rowsum, DMA out
  - Host: sum the 8 per-head partial outputs, undo the sort permutation.
"""

import sys
from contextlib import ExitStack

import numpy as np

for _p in ("/opt/trn_rl_repo", "/root/.axon_site/_ro/trn_rl_repo"):
    if _p not in sys.path:
        sys.path.append(_p)

import concourse.bacc as bacc
import concourse.mybir as mybir
import concourse.tile as tile
from concourse.bass_utils import run_bass_kernel_spmd
from concourse.masks import make_identity

B = 4096
DIM = 768
HEADS = 8
DH = 96
P = 128
KT = 6            # k-tiles over DIM for the qkv projections
NT = B // P       # 32 query tiles
NC = B // 512     # 8 column chunks for qkv production
CHUNKS_B1 = ((0, 1024), (1024, 1024), (2048, 1024), (3072, 1024))
CHUNKS_B2 = ((0, 1536), (1536, 1536), (3072, 1024))
NB1 = 8  # tiles processed with the B1 chunking (while phase A holds PSUM)

F32 = mybir.dt.float32
F32R = mybir.dt.float32r
BF16 = mybir.dt.bfloat16


def build_program(bands, max_njt, phases="AB", inline_map=None):
    nc = bacc.Bacc("TRN2", target_bir_lowering=False, debug=False)

    if inline_map is None:
        xT_d = nc.dram_tensor("xT", [DIM, B], F32, kind="ExternalInput").ap()
        wq_d = nc.dram_tensor("wq", [DIM, DH], F32, kind="ExternalInput").ap()
        wk_d = nc.dram_tensor("wk", [DIM, DH], F32, kind="ExternalInput").ap()
        wv_d = nc.dram_tensor("wv", [DIM, DH], F32, kind="ExternalInput").ap()
        wo_d = nc.dram_tensor("wo", [DH, DIM], F32, kind="ExternalInput").ap()
        lab_row_d = nc.dram_tensor("lab_row", [1, B], F32, kind="ExternalInput").ap()
        lab_col_d = nc.dram_tensor("lab_col", [P, NT], F32, kind="ExternalInput").ap()
        out_d = nc.dram_tensor("out", [B, DIM], F32, kind="ExternalOutput").ap()
        tick_d = None
    else:
        m = inline_map
        xT_d = nc.inline_tensor(m["xT"], "xT").ap()
        wq_d = nc.inline_tensor(m["wq"], "wq").ap()
        wk_d = nc.inline_tensor(m["wk"], "wk").ap()
        wv_d = nc.inline_tensor(m["wv"], "wv").ap()
        wo_d = nc.inline_tensor(m["wo"], "wo").ap()
        lab_row_d = nc.inline_tensor(m["lab_row"], "lab_row").ap()
        lab_col_d = nc.inline_tensor(m["lab_col"], "lab_col").ap()
        out_d = nc.dram_tensor("out_i", [B, DIM], F32).ap()
        tick_d = nc.dram_tensor("tick", [1, 32], F32, kind="ExternalOutput").ap()

    bandw = max_njt * P

    with tile.TileContext(nc, pool_alloc_mode="queue") as tc, ExitStack() as top:
        persist = top.enter_context(tc.tile_pool(name="persist", bufs=1))

        wq_sb = persist.tile([P, KT, DH], F32R)
        wk_sb = persist.tile([P, KT, DH], F32R)
        wv_sb = persist.tile([P, KT, DH], F32R)
        for w_sb, w_d in ((wq_sb, wq_d), (wk_sb, wk_d), (wv_sb, wv_d)):
            nc.sync.dma_start(
                w_sb[:], w_d.rearrange("(kt p) d -> p kt d", p=P).bitcast(F32R)
            )
        wo_sb = persist.tile([DH, DIM], F32R)
        lab_col_sb = persist.tile([P, NT], F32)
        lab_rep = persist.tile([P, B], F32)

        ident_bf = persist.tile([P, P], BF16)
        make_identity(nc, ident_bf)

        qT_sb = persist.tile([DH, B], BF16)
        kT_sb = persist.tile([DH, B], BF16)
        vn_sb = persist.tile([P, NT, DH], BF16)

        with (
            tc.tile_pool(name="attn", bufs=10) as attnpool,
            tc.tile_pool(name="small", bufs=10) as smallpool,
            tc.tile_pool(name="osb", bufs=2) as opool,
            tc.tile_pool(name="ps_m", bufs=1, space="PSUM") as mpool,
        ):
            with (
                tc.tile_pool(name="xa", bufs=12) as xpool,
                tc.tile_pool(name="vtc", bufs=2) as vtpool,
                tc.tile_pool(name="pa", bufs=2, space="PSUM") as apsum,
                tc.tile_pool(name="ps_s1", bufs=2, space="PSUM") as spool1,
            ):
                attns = {}
                accs = {}
                if "B" in phases:
                    for t in range(NB1):
                        attns[t] = attnpool.tile(
                            [P, B], BF16, tag="attn", name=f"attn{t}"
                        )
                        accs[t] = smallpool.tile(
                            [P, 4], F32, tag="acc", name=f"acc{t}"
                        )
                for c in range(NC):
                    if "A" in phases:
                        _phase_a_chunk(nc, tc, xpool, vtpool, apsum, mpool,
                                       xT_d, wq_sb, wk_sb, wv_sb, qT_sb,
                                       kT_sb, vn_sb, ident_bf, c)
                    if c == 4:
                        # deferred small input DMAs
                        nc.sync.dma_start(wo_sb[:], wo_d.bitcast(F32R))
                        nc.sync.dma_start(lab_col_sb[:], lab_col_d)
                        nc.sync.dma_start(
                            lab_rep[:], lab_row_d.to_broadcast([P, B])
                        )
                    if "B" in phases and c % 2 == 1:
                        ci = (c - 1) // 2
                        c0, cw = CHUNKS_B1[ci]
                        for t in range(NB1):
                            _dots_exp(nc, spool1, qT_sb, kT_sb, attns[t],
                                      accs[t], t, ci, c0, cw, 1024)
                if "B" in phases:
                    for t in range(NB1):
                        _tile_tail(nc, smallpool, opool, mpool, bands, bandw,
                                   vn_sb, lab_rep, lab_col_sb, wo_sb, ident_bf,
                                   out_d, attns[t], accs[t], t, len(CHUNKS_B1))
            if "B" in phases:
                with tc.tile_pool(name="ps_s2", bufs=2, space="PSUM") as spool2:
                    _phase_b(nc, tc, attnpool, smallpool, opool, spool2, mpool,
                             bands, bandw, qT_sb, kT_sb, vn_sb, lab_rep,
                             lab_col_sb, wo_sb, ident_bf, out_d,
                             trange=range(NB1, NT), chunks=CHUNKS_B2, sw=1536)

        if tick_d is not None:
            with tc.tile_pool(name="tickp", bufs=1) as tickp:
                tk = tickp.tile([1, 32], F32)
                nc.sync.dma_start(tk[:], out_d[B - 1 : B, 0:32])
                nc.sync.dma_start(tick_d[:], tk[:])

    nc.compile()
    return nc


def _phase_a_chunk(nc, tc, xpool, vtpool, apsum, a2psum,
                   xT_d, wq_sb, wk_sb, wv_sb, qT_sb, kT_sb, vn_sb, ident_bf,
                   c):
    """One 512-column chunk of the qkv projections (K, then Q, then V)."""
    if True:
        if True:
            cs = slice(c * 512, (c + 1) * 512)
            xts = []
            for kt in range(KT):
                xt = xpool.tile([P, 512], F32R, tag="xt", name=f"xt{c}_{kt}")
                nc.sync.dma_start(
                    xt[:], xT_d[kt * P : (kt + 1) * P, cs].bitcast(F32R)
                )
                xts.append(xt)
            for w_sb, dst in ((wk_sb, kT_sb), (wq_sb, qT_sb), (wv_sb, None)):
                ps = apsum.tile([DH, 512], F32, tag="qkv", name=f"qkvps{c}")
                for kt in range(KT):
                    nc.tensor.matmul(
                        ps[:],
                        w_sb[:, kt, :],
                        xts[kt][:],
                        start=(kt == 0),
                        stop=(kt == KT - 1),
                    )
                if dst is not None:
                    nc.vector.tensor_copy(dst[:, cs], ps[:])
                else:
                    vt_c = vtpool.tile([DH, 512], BF16, tag="vtc", name=f"vt{c}")
                    nc.vector.tensor_copy(vt_c[:], ps[:])
                    # V.T chunk -> V natural layout (bf16), 4 j-tiles
                    for j in range(4):
                        jt = c * 4 + j
                        tpv = a2psum.tile([P, DH], BF16, tag="m", name=f"vn{jt}")
                        nc.tensor.transpose(
                            tpv[:], vt_c[:, j * P : (j + 1) * P], ident_bf[:DH, :DH]
                        )
                        nc.vector.tensor_copy(vn_sb[:, jt, :], tpv[:])


def _dots_exp(nc, spool, qT_sb, kT_sb, attn, acc, t, ci, c0, cw, sw):
    s = spool.tile([P, sw], F32, tag="s", name=f"s{t}_{ci}")
    for mi in range(cw // 512):
        nc.tensor.matmul(
            s[:, mi * 512 : (mi + 1) * 512],
            qT_sb[:, t * P : (t + 1) * P],
            kT_sb[:, c0 + mi * 512 : c0 + (mi + 1) * 512],
        )
    nc.scalar.activation(
        out=attn[:, c0 : c0 + cw],
        in_=s[:, :cw],
        func=mybir.ActivationFunctionType.Exp,
        accum_out=acc[:, ci : ci + 1],
    )


def _phase_b(nc, tc, attnpool, smallpool, opool, spool, mpool,
             bands, bandw, qT_sb, kT_sb, vn_sb, lab_rep,
             lab_col_sb, wo_sb, ident_bf, out_d, trange, chunks, sw,
             colmajor=False):
    attns = {}
    accs = {}
    for t in trange:
        attns[t] = attnpool.tile([P, B], BF16, tag="attn", name=f"attn{t}")
        accs[t] = smallpool.tile([P, 4], F32, tag="acc", name=f"acc{t}")
        if not colmajor:
            for ci, (c0, cw) in enumerate(chunks):
                _dots_exp(nc, spool, qT_sb, kT_sb, attns[t], accs[t],
                          t, ci, c0, cw, sw)
            _tile_tail(nc, smallpool, opool, mpool, bands, bandw, vn_sb,
                       lab_rep, lab_col_sb, wo_sb, ident_bf, out_d,
                       attns[t], accs[t], t, len(chunks))
    if colmajor:
        for ci, (c0, cw) in enumerate(chunks):
            for t in trange:
                _dots_exp(nc, spool, qT_sb, kT_sb, attns[t], accs[t],
                          t, ci, c0, cw, sw)
        for t in trange:
            _tile_tail(nc, smallpool, opool, mpool, bands, bandw, vn_sb,
                       lab_rep, lab_col_sb, wo_sb, ident_bf, out_d,
                       attns[t], accs[t], t, len(chunks))


def _tile_tail(nc, smallpool, opool, mpool, bands, bandw, vn_sb, lab_rep,
               lab_col_sb, wo_sb, ident_bf, out_d, attn, acc, t, nchunks):
    if True:
        if True:
            jt0, njt = bands[t]
            bw = njt * P
            sums = smallpool.tile([P, 1], F32, tag="sums", name=f"sums{t}")
            nc.vector.tensor_reduce(
                out=sums[:],
                in_=acc[:, :nchunks],
                axis=mybir.AxisListType.X,
                op=mybir.AluOpType.add,
            )
            recip = smallpool.tile([P, 1], F32, tag="recip", name=f"recip{t}")
            nc.vector.reciprocal(recip[:], sums[:])

            # mask the band: attnm = (lab_j == lab_i) * attn
            attnm = smallpool.tile([P, bandw], BF16, tag="attnm", name=f"am{t}")
            nc.vector.scalar_tensor_tensor(
                out=attnm[:, :bw],
                in0=lab_rep[:, jt0 * P : jt0 * P + bw],
                scalar=lab_col_sb[:, t : t + 1],
                in1=attn[:, jt0 * P : jt0 * P + bw],
                op0=mybir.AluOpType.is_equal,
                op1=mybir.AluOpType.mult,
            )

            tp = mpool.tile([P, bandw], BF16, tag="m", name=f"tp{t}")
            for b in range(njt):
                nc.tensor.transpose(
                    tp[:, b * P : (b + 1) * P],
                    attnm[:, b * P : (b + 1) * P],
                    ident_bf[:],
                )
            attnT = smallpool.tile([P, bandw], BF16, tag="attnT", name=f"aT{t}")
            nc.vector.tensor_copy(attnT[:, :bw], tp[:, :bw])

            ot = mpool.tile([DH, P], F32, tag="m", name=f"ot{t}")
            for b in range(njt):
                nc.tensor.matmul(
                    ot[:],
                    vn_sb[:, jt0 + b, :],
                    attnT[:, b * P : (b + 1) * P],
                    start=(b == 0),
                    stop=(b == njt - 1),
                )
            ot_sb = smallpool.tile([DH, P], F32R, tag="ot_sb", name=f"os{t}")
            nc.vector.tensor_copy(ot_sb[:], ot[:])

            osb = opool.tile([P, DIM], F32, tag="osb", name=f"o{t}")
            fin = mpool.tile([P, DIM], F32, tag="m", name=f"fin{t}")
            nc.tensor.matmul(fin[:, 0:512], ot_sb[:], wo_sb[:, 0:512])
            nc.tensor.matmul(fin[:, 512:768], ot_sb[:], wo_sb[:, 512:768])
            nc.vector.tensor_scalar(
                out=osb[:],
                in0=fin[:],
                scalar1=recip[:],
                scalar2=None,
                op0=mybir.AluOpType.mult,
            )
            nc.sync.dma_start(out_d[t * P : (t + 1) * P, :], osb[:])


def _host_prep(x, labels, W_qkv, W_out):
    x = np.asarray(x, dtype=np.float32)
    labels = np.asarray(labels)
    W_qkv = np.asarray(W_qkv, dtype=np.float32)
    W_out = np.asarray(W_out, dtype=np.float32)

    perm = np.argsort(labels, kind="stable")
    ls = labels[perm]
    xT = np.ascontiguousarray(x[perm].T)

    lsf = ls.astype(np.float32)
    lab_row = np.ascontiguousarray(lsf.reshape(1, B))
    lab_col = np.ascontiguousarray(lsf.reshape(NT, P).T)

    bands = []
    for t in range(NT):
        jlo = int(np.searchsorted(ls, ls[t * P], side="left"))
        jhi = int(np.searchsorted(ls, ls[t * P + P - 1], side="right"))
        jt0 = jlo // P
        njt = (jhi - 1) // P - jt0 + 1
        bands.append((jt0, njt))
    max_njt = max(n for _, n in bands)

    scale = DH ** -0.5
    in_maps = []
    for h in range(HEADS):
        sl = slice(h * DH, (h + 1) * DH)
        in_maps.append(
            {
                "xT": xT,
                "wq": np.ascontiguousarray(W_qkv[:, sl]) * scale,
                "wk": np.ascontiguousarray(W_qkv[:, 768 + h * DH : 768 + (h + 1) * DH]),
                "wv": np.ascontiguousarray(
                    W_qkv[:, 1536 + h * DH : 1536 + (h + 1) * DH]
                ),
                "wo": np.ascontiguousarray(W_out[sl, :]),
                "lab_row": lab_row,
                "lab_col": lab_col,
            }
        )
    return in_maps, bands, max_njt, perm


_CACHE = {}


def kernel(x, labels, W_qkv, W_out, _trace=False):
    in_maps, bands, max_njt, perm = _host_prep(x, labels, W_qkv, W_out)

    key = (tuple(bands), max_njt)
    if key not in _CACHE:
        _CACHE[key] = build_program(bands, max_njt)
    nc = _CACHE[key]

    res = run_bass_kernel_spmd(
        nc, in_maps, core_ids=list(range(HEADS)), trace=_trace
    )
    acc = np.zeros((B, DIM), dtype=np.float64)
    for c in range(HEADS):
        acc += res.results[c]["out"].astype(np.float64)

    out = np.empty((B, DIM), dtype=np.float32)
    out[perm] = acc.astype(np.float32)
    if _trace:
        return out, res
    return out
